# revision 1
# baseline (speedup 1.0000x reference)
"""CIN (nn_CIN_35450660061557) Bass/Tile kernel for 8 TRN2 NeuronCores.

Math (per batch b, embed position d — each (b,d) "column" is independent):
  h_{l+1}[o] = relu( sum_{h,m} Wr_l[o,h,m] * h_l[h] * x0[m] + b_l[o] )
  score[b]   = lb + sum_{l,o,d} lw_l[o] * h_l[o, (b,d)]

Mapping (v3):
  - Data-parallel over batch: 8 cores x 64 batches; N = 64*64 = 4096
    columns/core, as 4 column-pairs of 1024 = 2 halves of 512.
  - Layer 0 uses the symmetry of z0 = x0 (x) x0: folded weights
    W0f[(a,b)] = W0[a,b]+W0[b,a] (a<b) cut K from 1024 to 640 (5
    chunks); z0f precomputed on host, streamed as 128 x 10KB
    descriptors per pair.
  - x0 column-broadcast: host stores x0 tile-major so each broadcast
    DMA is 128 x 16KB contiguous descriptors.
  - h is evacuated from PSUM by the ACT engine four times per column
    half into hrep[128, 4, 512], so every DVE z-multiply is a plain
    strided bf16 tensor_tensor hitting the 2x_1P perf mode.
  - Half-major matmul order: all 32 half-A matmuls, then half-A evac
    (4 RELUs) overlapping the 32 half-B matmuls — keeps the serial
    RELU chain off the critical path.
  - GPSIMD produces the half-B z of groups 2/5/7 (mid-layer deadlines
    with ~2us slack; emitted at layer start).
  - Scores fold lw into M=1 matmuls PSUM-accumulated across layers;
    DVE reduces over d on-chip: one [1, 64] fp32 DMA per core.
  - Small/constant DMAs ride the ACT HWDGE queue so the first-pair
    bias load is not stuck behind 23us of xb broadcast on SP.
"""

import numpy as np
import ml_dtypes

B, M, D = 512, 32, 64
O = 128                      # layer width (all 3 layers)
NCORES = 8
BL = B // NCORES             # 64 batches per core
N = BL * D                   # 4096 columns per core
PW = 1024                    # columns per pair
NP = N // PW                 # 4 pairs per core
NT = 512                     # columns per half / matmul moving width
G0F = 5                      # folded layer-0 K chunks (640 rows)
K0F = G0F * 128
G = 32                       # layer-1/2 K chunks (m index)
GP_GRPS = (3, 6)             # groups whose half-B z comes from GPSIMD
BF16 = ml_dtypes.bfloat16

_CACHE = {}


def _fold_pairs():
    """Upper-triangle (a<=b) pair enumeration for the symmetric z0."""
    ia, ib = np.triu_indices(M)
    return ia.astype(np.int64), ib.astype(np.int64)  # 528 pairs


def _build():
    from contextlib import ExitStack

    import concourse.bass as bass
    import concourse.mybir as mybir
    import concourse.tile as tile
    from concourse import bacc

    fp32 = mybir.dt.float32
    bf16 = mybir.dt.bfloat16
    Relu = mybir.ActivationFunctionType.Relu
    Add = mybir.AluOpType.add
    AxX = mybir.AxisListType.X

    nc = bacc.Bacc("TRN2", target_bir_lowering=False, debug=False)

    # xc rows: r = tile*2 + rowhalf -> (16 m-rows x 512 cols) contiguous
    xc_d = nc.dram_tensor("xc", [16, 16 * NT], bf16, kind="ExternalInput").ap()
    # z0 rows: pair*128 + k; per row G0F x 1024 cols contiguous
    z0_d = nc.dram_tensor("z0", [NP * 128, G0F * PW], bf16, kind="ExternalInput").ap()
    w0_d = nc.dram_tensor("w0p", [128, G0F * O], bf16, kind="ExternalInput").ap()
    w1_d = nc.dram_tensor("w1p", [128, G * O], bf16, kind="ExternalInput").ap()
    w2_d = nc.dram_tensor("w2p", [128, G * O], bf16, kind="ExternalInput").ap()
    b0_d = nc.dram_tensor("b0", [O, 1], fp32, kind="ExternalInput").ap()
    b1_d = nc.dram_tensor("b1", [O, 1], fp32, kind="ExternalInput").ap()
    b2_d = nc.dram_tensor("b2", [O, 1], fp32, kind="ExternalInput").ap()
    lw_d = nc.dram_tensor("lwseg", [O, 3], bf16, kind="ExternalInput").ap()
    out_d = nc.dram_tensor("out", [1, BL], fp32, kind="ExternalOutput").ap()

    with tile.TileContext(nc) as tc, ExitStack() as ctx:
        const = ctx.enter_context(tc.tile_pool(name="const", bufs=1))
        xbp = ctx.enter_context(tc.tile_pool(name="xbp", bufs=5))
        z0p = ctx.enter_context(tc.tile_pool(name="z0p", bufs=2))
        zqp = ctx.enter_context(tc.tile_pool(name="zqp", bufs=10))
        zgp = ctx.enter_context(tc.tile_pool(name="zgp", bufs=4))
        hrp = ctx.enter_context(tc.tile_pool(name="hrp", bufs=6))
        h3p = ctx.enter_context(tc.tile_pool(name="h3p", bufs=4))
        psp = ctx.enter_context(tc.tile_pool(name="psp", bufs=3, space="PSUM"))
        pssp = ctx.enter_context(tc.tile_pool(name="pssp", bufs=2, space="PSUM"))

        def load_z0(p):
            # z0 rides the ACT HWDGE queue: after startup that queue is
            # empty, so layer-0's stream never waits behind xb broadcasts
            # (and neither does the batched DMA-completion semaphore the
            # first layer-0 matmul waits on).
            z0t = z0p.tile([128, G0F, PW], bf16, name=f"z0t{p}", tag="z0")
            nc.scalar.dma_start(
                out=z0t,
                in_=z0_d[bass.ts(p, 128)].rearrange("k (g c) -> k g c", c=PW),
            )
            return z0t

        def load_xb(t, rh):
            # xb[p, ml, c] = x0[rh*16 + ml, t*512 + c] for all 128 p
            xb = xbp.tile([128, 16, NT], bf16, name=f"xb{t}_{rh}", tag="xb")
            nc.sync.dma_start(
                out=xb,
                in_=xc_d[2 * t + rh : 2 * t + rh + 1]
                .rearrange("o (m c) -> o m c", c=NT)
                .partition_broadcast(128),
            )
            return xb

        # ---- constants: small loads on the ACT HWDGE queue so they are
        # not serialized behind the first pair's big SP-queue streams ----
        ball = const.tile([O, 3], fp32)
        lws = const.tile([O, 3], bf16)
        nc.scalar.dma_start(out=ball[:, 0:1], in_=b0_d)
        nc.scalar.dma_start(out=ball[:, 1:2], in_=b1_d)
        nc.scalar.dma_start(out=ball[:, 2:3], in_=b2_d)
        nc.scalar.dma_start(out=lws, in_=lw_d)
        w0s = const.tile([128, G0F, O], bf16)
        w1s = const.tile([128, G, O], bf16)
        w2s = const.tile([128, G, O], bf16)
        nc.scalar.dma_start(out=w0s, in_=w0_d.rearrange("k (g o) -> k g o", o=O))
        z0_0 = load_z0(0)
        out_asm = const.tile([1, BL], fp32)
        pre = {0: (z0_0, None)}

        def evac_half(ps, li, p, half):
            # 4 copies of this half of h so DVE z-multiplies have
            # unit-stride (non-broadcast) operands; ACT is cheap.
            hr = hrp.tile([128, 4, NT], bf16, tag="hr", name=f"hr{li}_{p}_{half}")
            for j in range(4):
                nc.scalar.activation(hr[:, j], ps, Relu, bias=ball[:, li : li + 1])
            return hr

        def ps_half(tag, name):
            return psp.tile([128, NT], fp32, tag=tag, name=name)

        def emit_l0(p):
            z0t = pre[p][0]
            hrs = []
            for half in range(2):
                cs = bass.ts(half, NT)
                ps0 = ps_half(f"ps{half}", f"ps0_{p}_{half}")
                for g in range(G0F):
                    nc.tensor.matmul(
                        ps0, w0s[:, g], z0t[:, g, cs],
                        start=(g == 0), stop=(g == G0F - 1),
                    )
                hrs.append(evac_half(ps0, 0, p, half))
            return hrs  # [hrA, hrB]

        def emit_layer(p, li, hrA, hrB, wls, last):
            xb = pre[p][1]  # [colhalf*2 + rowhalf]
            hr_in = (hrA, hrB)

            def zfill(eng, zt, grp, half):
                # z[(m,h),c] for m in [4*grp, 4*grp+4), cols half*512+[0,512)
                ms = bass.ts(grp % 4, 4)
                eng.tensor_mul(zt, hr_in[half], xb[2 * half + grp // 4][:, ms])

            # GPSIMD half-B z for groups 2/5/7 (mid-layer deadlines).
            zgB = {}
            for grp in GP_GRPS:
                zgB[grp] = zgp.tile(
                    [128, 4, NT], bf16, tag="zg", name=f"zg{grp}_{li}_{p}"
                )
                zfill(nc.gpsimd, zgB[grp], grp, 1)

            outs = []
            for half in range(2):
                ps = ps_half(f"ps{half}", f"ps{li + 1}_{p}_{half}")
                for grp in range(8):
                    if half == 1 and grp in zgB:
                        zt = zgB[grp]
                    else:
                        zt = zqp.tile(
                            [128, 4, NT], bf16, tag="zq",
                            name=f"zq{grp}_{li}_{p}_{half}",
                        )
                        zfill(nc.vector, zt, grp, half)
                    for j in range(4):
                        k = 4 * grp + j
                        nc.tensor.matmul(
                            ps, wls[:, k], zt[:, j],
                            start=(k == 0), stop=(k == G - 1),
                        )
                if last:
                    h3 = h3p.tile([128, NT], bf16, tag="h3", name=f"h3_{p}_{half}")
                    nc.scalar.activation(h3, ps, Relu, bias=ball[:, li + 1 : li + 2])
                    outs.append(h3)
                else:
                    outs.append(evac_half(ps, li + 1, p, half))
            return outs

        def emit_score(p, hs1, hs2, hs3):
            for half in range(2):
                pss = pssp.tile([1, NT], fp32, tag="pss")
                nc.tensor.matmul(
                    pss, lws[:, 0:1], hs1[half][:, 0], start=True, stop=False
                )
                nc.tensor.matmul(
                    pss, lws[:, 1:2], hs2[half][:, 0], start=False, stop=False
                )
                nc.tensor.matmul(
                    pss, lws[:, 2:3], hs3[half], start=False, stop=True
                )
                bs = 16 * p + 8 * half
                nc.vector.tensor_reduce(
                    out=out_asm[0:1, bs : bs + 8],
                    in_=pss.rearrange("o (b d) -> o b d", d=D),
                    axis=AxX,
                    op=Add,
                )

        def load_pair(p):
            z0t = load_z0(p)
            xbs = [load_xb(2 * p, 0), load_xb(2 * p, 1), load_xb(2 * p + 1, 0),
                   load_xb(2 * p + 1, 1)]
            return (z0t, xbs)

        # Software-pipelined emission: pair p+1's DMA prefetch and
        # layer-0 are emitted between pair p's layer-1 and layer-2, and
        # pair p's scores are emitted one iteration LATE (mid pair p+1)
        # so the score matmuls + DVE reduces never head-of-line-block
        # the next pair's z fills on the in-order engine queues.
        # Pair 0's layer-0 is emitted BEFORE the first xb broadcasts so
        # the scheduler's batched DMA-completion wait on its first
        # matmul covers only z0+weights (~10us), not 23us of xb.
        h1s = {0: emit_l0(0)}
        xb_0 = [load_xb(0, 0), load_xb(0, 1), load_xb(1, 0), load_xb(1, 1)]
        nc.scalar.dma_start(out=w1s, in_=w1_d.rearrange("k (g o) -> k g o", o=O))
        nc.scalar.dma_start(out=w2s, in_=w2_d.rearrange("k (g o) -> k g o", o=O))
        pre[0] = (z0_0, xb_0)
        scoreq = {}
        for p in range(NP):
            hs1 = h1s.pop(p)
            if p - 1 in scoreq:
                emit_score(p - 1, *scoreq.pop(p - 1))
            hs2 = emit_layer(p, 0, hs1[0], hs1[1], w1s, last=False)
            if p + 1 < NP:
                pre[p + 1] = load_pair(p + 1)
                h1s[p + 1] = emit_l0(p + 1)
            hs3 = emit_layer(p, 1, hs2[0], hs2[1], w2s, last=True)
            scoreq[p] = (hs1, hs2, hs3)
            del pre[p]
        emit_score(NP - 1, *scoreq.pop(NP - 1))

        nc.scalar.dma_start(out=out_d, in_=out_asm)

    nc.compile()
    return nc


def prep_inputs(**inputs):
    """Host-side prep: per-core input maps (shard batch, permute weights)."""
    inp = np.asarray(inputs["input"], np.float32)
    W0 = np.asarray(inputs["W0"], np.float32)
    W1 = np.asarray(inputs["W1"], np.float32)
    W2 = np.asarray(inputs["W2"], np.float32)
    lw = np.asarray(inputs["lw"], np.float32)

    # Layers 1/2: WpT[(m*H+h), o] = Wr[o, h, m]; SBUF layout [k, (g, o)]
    # with chunk g == m (128 h-rows per chunk).
    def _prep_w(W, H):
        wp = W.reshape(O, H, M).transpose(2, 1, 0).reshape(H * M, O)
        g = H * M // 128
        return np.ascontiguousarray(
            wp.reshape(g, 128, O).transpose(1, 0, 2).reshape(128, g * O)
        ).astype(BF16)

    # Layer 0 folded: K index = upper-tri pair (a<=b); weight
    # W0f[o, (a,b)] = Wr0[o,a,b] + Wr0[o,b,a] (a<b), Wr0[o,a,a] (diag).
    ia, ib = _fold_pairs()
    Wr0 = W0.reshape(O, M, M)
    w0f = Wr0[:, ia, ib] + np.where(ia != ib, 1.0, 0.0)[None, :] * Wr0[:, ib, ia]
    w0f = np.concatenate(
        [w0f, np.zeros((O, K0F - w0f.shape[1]), np.float32)], axis=1
    )  # [O, 640]
    w0p = np.ascontiguousarray(
        w0f.T.reshape(G0F, 128, O).transpose(1, 0, 2).reshape(128, G0F * O)
    ).astype(BF16)

    w1p = _prep_w(W1, O)
    w2p = _prep_w(W2, O)
    b0 = np.asarray(inputs["b0"], np.float32).reshape(O, 1)
    b1 = np.asarray(inputs["b1"], np.float32).reshape(O, 1)
    b2 = np.asarray(inputs["b2"], np.float32).reshape(O, 1)
    lwseg = np.ascontiguousarray(lw.reshape(3, O).T).astype(BF16)

    shared = dict(w0p=w0p, w1p=w1p, w2p=w2p, b0=b0, b1=b1, b2=b2, lwseg=lwseg)
    in_maps = []
    for c in range(NCORES):
        xcore = np.ascontiguousarray(
            inp[BL * c : BL * (c + 1)].transpose(1, 0, 2).reshape(M, N)
        ).astype(BF16)
        # xc tile-major: row r = tile*2 + rowhalf -> 16 m-rows x 512 cols
        xc = np.ascontiguousarray(
            xcore.reshape(2, 16, 8, NT).transpose(2, 0, 1, 3).reshape(16, 16 * NT)
        )
        xf = xcore.astype(np.float32)
        z0f = (xf[ia] * xf[ib]).astype(BF16)  # [528, N]
        z0f = np.concatenate([z0f, np.zeros((K0F - z0f.shape[0], N), BF16)], axis=0)
        # z0 DMA layout: row (pair*128 + k), per row chunks g x 1024 cols
        z0 = np.ascontiguousarray(
            z0f.reshape(G0F, 128, NP, PW).transpose(2, 1, 0, 3).reshape(NP * 128, G0F * PW)
        )
        in_maps.append(dict(shared, xc=xc, z0=z0))
    return in_maps


def kernel(**inputs):
    import os

    from concourse import bass_utils

    if "nc" not in _CACHE:
        _CACHE["nc"] = _build()
    nc = _CACHE["nc"]

    in_maps = prep_inputs(**inputs)
    trace = os.environ.get("CIN_TRACE") == "1"
    res = bass_utils.run_bass_kernel_spmd(
        nc, in_maps, core_ids=list(range(NCORES)), trace=trace
    )
    _CACHE["last_res"] = res
    lb = float(np.asarray(inputs["lb"], np.float32).reshape(-1)[0])
    out = np.concatenate(
        [res.results[c]["out"].astype(np.float32).reshape(BL) for c in range(NCORES)]
    )
    return out + lb



# revision 2
# speedup vs baseline: 1.0169x; 1.0169x over previous
"""CIN (nn_CIN_35450660061557) Bass/Tile kernel for 8 TRN2 NeuronCores. v4

Math (per batch b, embed position d — each (b,d) "column" is independent):
  h_{l+1}[o] = relu( sum_{h,m} Wr_l[o,h,m] * h_l[h] * x0[m] + b_l[o] )
  score[b]   = lb + sum_{l,o,d} lw_l[o] * h_l[o, (b,d)]

Mapping (v4):
  - Data-parallel over batch: 8 cores x 64 batches; N = 64*64 = 4096
    columns/core, as 4 column-pairs of 1024 = 2 halves of 512.
  - Layer 0: host-folded symmetric z0 (K=640), streamed per half on the
    ACT HWDGE queue.
  - z-fill for layers 1/2 uses broadcast (stride-0) h operands — DVE
    tensor_tensor keeps the 2x_1P perf mode with a stride-0 outer dim,
    so h is evacuated ONCE per half (no 4x replication) and fills are
    emitted as few big ops: DVE [128,8,512] (2.2us) and one GPSIMD
    [128,12,512] (11.7us) per pair-layer whose output is consumed by
    the LAST 12 matmuls of the layer (longest deadline).
  - x0 column-broadcast unchanged: host stores x0 tile-major; each
    broadcast DMA is 128 x 16KB contiguous descriptors on the SP queue.
  - Software-pipelined pairs as in v3: pair p+1's loads + layer-0 are
    emitted between pair p's layer-1 and layer-2; scores one pair late.
"""

import numpy as np
import ml_dtypes

B, M, D = 512, 32, 64
O = 128                      # layer width (all 3 layers)
NCORES = 8
BL = B // NCORES             # 64 batches per core
N = BL * D                   # 4096 columns per core
PW = 1024                    # columns per pair
NP = N // PW                 # 4 pairs per core
NT = 512                     # columns per half / matmul moving width
G0F = 5                      # folded layer-0 K chunks (640 rows)
K0F = G0F * 128
G = 32                       # layer-1/2 K chunks (m index)
GP_W = 12                    # m-chunks per pair-layer filled by GPSIMD
BF16 = ml_dtypes.bfloat16

_CACHE = {}


def _fold_pairs():
    """Upper-triangle (a<=b) pair enumeration for the symmetric z0."""
    ia, ib = np.triu_indices(M)
    return ia.astype(np.int64), ib.astype(np.int64)  # 528 pairs


def _build():
    from contextlib import ExitStack

    import concourse.bass as bass
    import concourse.mybir as mybir
    import concourse.tile as tile
    from concourse import bacc

    fp32 = mybir.dt.float32
    bf16 = mybir.dt.bfloat16
    Relu = mybir.ActivationFunctionType.Relu
    Add = mybir.AluOpType.add
    AxX = mybir.AxisListType.X

    nc = bacc.Bacc("TRN2", target_bir_lowering=False, debug=False)

    # xc rows: r = tile*2 + rowhalf -> (16 m-rows x 512 cols) contiguous
    xc_d = nc.dram_tensor("xc", [16, 16 * NT], bf16, kind="ExternalInput").ap()
    # z0 rows: pair*128 + k; per row G0F x 1024 cols contiguous
    z0_d = nc.dram_tensor("z0", [NP * 128, G0F * PW], bf16, kind="ExternalInput").ap()
    w0_d = nc.dram_tensor("w0p", [128, G0F * O], bf16, kind="ExternalInput").ap()
    w1_d = nc.dram_tensor("w1p", [128, G * O], bf16, kind="ExternalInput").ap()
    w2_d = nc.dram_tensor("w2p", [128, G * O], bf16, kind="ExternalInput").ap()
    b0_d = nc.dram_tensor("b0", [O, 1], fp32, kind="ExternalInput").ap()
    b1_d = nc.dram_tensor("b1", [O, 1], fp32, kind="ExternalInput").ap()
    b2_d = nc.dram_tensor("b2", [O, 1], fp32, kind="ExternalInput").ap()
    lw_d = nc.dram_tensor("lwseg", [O, 3], bf16, kind="ExternalInput").ap()
    out_d = nc.dram_tensor("out", [1, BL], fp32, kind="ExternalOutput").ap()

    with tile.TileContext(nc) as tc, ExitStack() as ctx:
        const = ctx.enter_context(tc.tile_pool(name="const", bufs=1))
        xbp = ctx.enter_context(tc.tile_pool(name="xbp", bufs=5))
        z0p = ctx.enter_context(tc.tile_pool(name="z0p", bufs=3))
        zqp = ctx.enter_context(tc.tile_pool(name="zqp", bufs=9))
        hrp = ctx.enter_context(tc.tile_pool(name="hrp", bufs=10))
        h3p = ctx.enter_context(tc.tile_pool(name="h3p", bufs=4))
        psp = ctx.enter_context(tc.tile_pool(name="psp", bufs=4, space="PSUM"))
        pssp = ctx.enter_context(tc.tile_pool(name="pssp", bufs=2, space="PSUM"))

        def load_z0(p, half):
            # z0 rides the ACT HWDGE queue (empty after startup).
            z0t = z0p.tile([128, G0F, NT], bf16, name=f"z0t{p}_{half}", tag="z0")
            nc.scalar.dma_start(
                out=z0t,
                in_=z0_d[bass.ts(p, 128)].rearrange(
                    "k (h g c) -> k h g c", h=2, c=NT
                )[:, half],
            )
            return z0t

        def load_xb(t, rh):
            # xb[p, ml, c] = x0[rh*16 + ml, t*512 + c] for all 128 p
            xb = xbp.tile([128, 16, NT], bf16, name=f"xb{t}_{rh}", tag="xb")
            nc.sync.dma_start(
                out=xb,
                in_=xc_d[2 * t + rh : 2 * t + rh + 1]
                .rearrange("o (m c) -> o m c", c=NT)
                .partition_broadcast(128),
            )
            return xb

        # ---- constants on the ACT HWDGE queue ----
        ball = const.tile([O, 3], fp32)
        lws = const.tile([O, 3], bf16)
        nc.scalar.dma_start(out=ball[:, 0:1], in_=b0_d)
        nc.scalar.dma_start(out=ball[:, 1:2], in_=b1_d)
        nc.scalar.dma_start(out=ball[:, 2:3], in_=b2_d)
        nc.scalar.dma_start(out=lws, in_=lw_d)
        w0s = const.tile([128, G0F, O], bf16)
        w1s = const.tile([128, G, O], bf16)
        w2s = const.tile([128, G, O], bf16)
        nc.scalar.dma_start(out=w0s, in_=w0_d.rearrange("k (g o) -> k g o", o=O))
        z0_0 = [load_z0(0, 0), load_z0(0, 1)]
        out_asm = const.tile([1, BL], fp32)

        def evac(ps, li, name):
            hr = hrp.tile([128, NT], bf16, tag="hr", name=name)
            nc.scalar.activation(hr, ps, Relu, bias=ball[:, li : li + 1])
            return hr

        def emit_l0(p, z0t):
            hs = []
            for half in range(2):
                ps = psp.tile([128, NT], fp32, tag="ps", name=f"ps0_{p}_{half}")
                for g in range(G0F):
                    nc.tensor.matmul(
                        ps, w0s[:, g], z0t[half][:, g],
                        start=(g == 0), stop=(g == G0F - 1),
                    )
                hs.append(evac(ps, 0, f"h1_{p}_{half}"))
            return hs  # [hA, hB]

        def fill(eng, pool, tag, h_t, xb_t, mlo, n, name):
            zt = pool.tile([128, n, NT], bf16, tag=tag, name=name)
            eng.tensor_mul(
                zt,
                h_t.unsqueeze(1).broadcast_to([128, n, NT]),
                xb_t[:, mlo : mlo + n],
            )
            return zt

        def emit_fills(p, lc, half, h_t, xb):
            # z fills for layer lc, columns of `half`; consumption order
            # list of (ztile, m_start, n). All-DVE [128,8,512] ops.
            out = []
            for rh in range(2):
                xb_t = xb[2 * half + rh]
                for i in range(2):
                    zt = fill(nc.vector, zqp, "zq", h_t, xb_t, 8 * i, 8,
                              f"zq{lc}_{p}_{half}_{rh}_{i}")
                    out.append((zt, 16 * rh + 8 * i, 8))
            return out

        def emit_mm_layer(p, li, zfills, wls, last, after_half=None):
            # li: consuming layer index (1 or 2); zfills[half] = fill list
            outs = []
            for half in range(2):
                ps = psp.tile([128, NT], fp32, tag="ps", name=f"ps{li}_{p}_{half}")
                for zt, m0, n in zfills[half]:
                    for j in range(n):
                        k = m0 + j
                        nc.tensor.matmul(
                            ps, wls[:, k], zt[:, j],
                            start=(k == 0), stop=(k == G - 1),
                        )
                if last:
                    h3 = h3p.tile([128, NT], bf16, tag="h3", name=f"h3_{p}_{half}")
                    nc.scalar.activation(h3, ps, Relu, bias=ball[:, li : li + 1])
                    outs.append(h3)
                else:
                    outs.append(evac(ps, li, f"h{li + 1}_{p}_{half}"))
                if after_half is not None:
                    after_half(half, outs[half])
            return outs

        def emit_score(p, hs1, hs2, hs3):
            for half in range(2):
                pss = pssp.tile([1, NT], fp32, tag="pss")
                nc.tensor.matmul(
                    pss, lws[:, 0:1], hs1[half], start=True, stop=False
                )
                nc.tensor.matmul(
                    pss, lws[:, 1:2], hs2[half], start=False, stop=False
                )
                nc.tensor.matmul(
                    pss, lws[:, 2:3], hs3[half], start=False, stop=True
                )
                bs = 16 * p + 8 * half
                nc.vector.tensor_reduce(
                    out=out_asm[0:1, bs : bs + 8],
                    in_=pss.rearrange("o (b d) -> o b d", d=D),
                    axis=AxX,
                    op=Add,
                )

        def load_pair_xb(p):
            return [load_xb(2 * p, 0), load_xb(2 * p, 1), load_xb(2 * p + 1, 0),
                    load_xb(2 * p + 1, 1)]

        # ---- software-pipelined emission ----
        # Pair 0's layer-0 is emitted BEFORE the first xb broadcasts so
        # its first matmul's batched DMA wait covers only z0+weights.
        h1s = {0: emit_l0(0, z0_0)}
        xbs = {0: load_pair_xb(0)}
        nc.scalar.dma_start(out=w1s, in_=w1_d.rearrange("k (g o) -> k g o", o=O))
        nc.scalar.dma_start(out=w2s, in_=w2_d.rearrange("k (g o) -> k g o", o=O))
        z1f = {
            0: [emit_fills(0, 1, half, h1s[0][half], xbs[0]) for half in range(2)]
        }
        scoreq = {}
        for p in range(NP):
            hs1 = h1s.pop(p)
            if p - 1 in scoreq:
                emit_score(p - 1, *scoreq.pop(p - 1))

            # L1(p); after each half's evac, emit that half's z2 fills
            z2f = {}

            def after_l1_half(half, h_t, p=p):
                z2f[half] = emit_fills(p, 2, half, h_t, xbs[p])

            hs2 = emit_mm_layer(p, 1, z1f.pop(p), w1s, last=False,
                                after_half=after_l1_half)

            if p + 1 < NP:
                z0_n = [load_z0(p + 1, 0), load_z0(p + 1, 1)]
                xbs[p + 1] = load_pair_xb(p + 1)
                h1s[p + 1] = emit_l0(p + 1, z0_n)
                z1f[p + 1] = [
                    emit_fills(p + 1, 1, half, h1s[p + 1][half], xbs[p + 1])
                    for half in range(2)
                ]

            hs3 = emit_mm_layer(p, 2, [z2f[0], z2f[1]], w2s, last=True)
            scoreq[p] = (hs1, hs2, hs3)
            del xbs[p]
        emit_score(NP - 1, *scoreq.pop(NP - 1))

        nc.scalar.dma_start(out=out_d, in_=out_asm)

    nc.compile()
    return nc


def prep_inputs(**inputs):
    """Host-side prep: per-core input maps (shard batch, permute weights)."""
    inp = np.asarray(inputs["input"], np.float32)
    W0 = np.asarray(inputs["W0"], np.float32)
    W1 = np.asarray(inputs["W1"], np.float32)
    W2 = np.asarray(inputs["W2"], np.float32)
    lw = np.asarray(inputs["lw"], np.float32)

    # Layers 1/2: WpT[(m*H+h), o] = Wr[o, h, m]; SBUF layout [k, (g, o)]
    # with chunk g == m (128 h-rows per chunk).
    def _prep_w(W, H):
        wp = W.reshape(O, H, M).transpose(2, 1, 0).reshape(H * M, O)
        g = H * M // 128
        return np.ascontiguousarray(
            wp.reshape(g, 128, O).transpose(1, 0, 2).reshape(128, g * O)
        ).astype(BF16)

    # Layer 0 folded: K index = upper-tri pair (a<=b); weight
    # W0f[o, (a,b)] = Wr0[o,a,b] + Wr0[o,b,a] (a<b), Wr0[o,a,a] (diag).
    ia, ib = _fold_pairs()
    Wr0 = W0.reshape(O, M, M)
    w0f = Wr0[:, ia, ib] + np.where(ia != ib, 1.0, 0.0)[None, :] * Wr0[:, ib, ia]
    w0f = np.concatenate(
        [w0f, np.zeros((O, K0F - w0f.shape[1]), np.float32)], axis=1
    )  # [O, 640]
    w0p = np.ascontiguousarray(
        w0f.T.reshape(G0F, 128, O).transpose(1, 0, 2).reshape(128, G0F * O)
    ).astype(BF16)

    w1p = _prep_w(W1, O)
    w2p = _prep_w(W2, O)
    b0 = np.asarray(inputs["b0"], np.float32).reshape(O, 1)
    b1 = np.asarray(inputs["b1"], np.float32).reshape(O, 1)
    b2 = np.asarray(inputs["b2"], np.float32).reshape(O, 1)
    lwseg = np.ascontiguousarray(lw.reshape(3, O).T).astype(BF16)

    shared = dict(w0p=w0p, w1p=w1p, w2p=w2p, b0=b0, b1=b1, b2=b2, lwseg=lwseg)
    in_maps = []
    for c in range(NCORES):
        xcore = np.ascontiguousarray(
            inp[BL * c : BL * (c + 1)].transpose(1, 0, 2).reshape(M, N)
        ).astype(BF16)
        # xc tile-major: row r = tile*2 + rowhalf -> 16 m-rows x 512 cols
        xc = np.ascontiguousarray(
            xcore.reshape(2, 16, 8, NT).transpose(2, 0, 1, 3).reshape(16, 16 * NT)
        )
        xf = xcore.astype(np.float32)
        z0f = (xf[ia] * xf[ib]).astype(BF16)  # [528, N]
        z0f = np.concatenate([z0f, np.zeros((K0F - z0f.shape[0], N), BF16)], axis=0)
        # z0 DMA layout: row (pair*128 + k), per row chunks g x 1024 cols
        z0 = np.ascontiguousarray(
            z0f.reshape(G0F, 128, NP, 2, NT)
            .transpose(2, 1, 3, 0, 4)
            .reshape(NP * 128, 2 * G0F * NT)
        )
        in_maps.append(dict(shared, xc=xc, z0=z0))
    return in_maps


def kernel(**inputs):
    import os

    from concourse import bass_utils

    if "nc" not in _CACHE:
        _CACHE["nc"] = _build()
    nc = _CACHE["nc"]

    in_maps = prep_inputs(**inputs)
    trace = os.environ.get("CIN_TRACE") == "1"
    res = bass_utils.run_bass_kernel_spmd(
        nc, in_maps, core_ids=list(range(NCORES)), trace=trace
    )
    _CACHE["last_res"] = res
    lb = float(np.asarray(inputs["lb"], np.float32).reshape(-1)[0])
    out = np.concatenate(
        [res.results[c]["out"].astype(np.float32).reshape(BL) for c in range(NCORES)]
    )
    return out + lb
